# revision 8
# baseline (speedup 1.0000x reference)
"""Distributed Trainium2 kernel for causal GQA attention (llama-style).

Reference computation (B=2, S=2048, D=2048, H=32 q-heads, KV=8 kv-heads,
HD=64, f32):
    q = rope(x @ wq.T), k = rope(x @ wk.T), v = x @ wv.T
    out = causal_softmax(q k^T / 8) v        (GQA: 4 q-heads per kv head)
    y = out @ wo.T

Sharding: tensor-parallel over heads — core c owns q-heads [4c, 4c+4) and
kv-head c, so each KV group stays local. The attention output is exchanged
with a single AllToAll (scatter tokens / gather heads), after which each
core runs the full wo projection on its 512-token shard. The host
concatenates the 8 token shards — no all-reduce anywhere.

Device-side tricks:
  - All inputs pre-transposed/cast to bf16 on the host; x is passed as
    x^T [D, B*S] so the contraction dim lands on SBUF partitions.
  - RoPE: wq/wk rows are permuted per-head on the host (even indices
    first), turning interleaved rope into rotate-half form = two
    tensor-tensor multiplies + add with precomputed [128, T] cos/sin
    tables (the 32-row block swap is done with SBUF-to-SBUF DMAs).
  - Scores are computed transposed (S^T[kt, qt]) so the PV matmul needs
    no transposes; the two heads of a 128-row q tile are computed
    concurrently with row-tiled matmuls (tile_position (0,0)/(64,0))
    against a duplicated [k; k] weight tile.
  - Softmax runs without max-subtraction (scores are O(5) by
    construction); causality is structural (upper tiles skipped) plus a
    binary mask multiply on diagonal tiles after exp.
  - The softmax denominator falls out of the PV matmul via a ones column
    appended to v (M=65); normalization happens once at the end via a
    batched reciprocal + rank-1 broadcast matmuls.
"""

import sys

if "/opt/trn_rl_repo" not in sys.path:
    sys.path.insert(0, "/opt/trn_rl_repo")

import numpy as np
import ml_dtypes

B, S, D = 2, 2048, 2048
H, KV, HD = 32, 8, 64
NCORES = 8
T = B * S  # 4096 flattened tokens
TSH = T // NCORES  # 512-token output shard per core
NKT = S // 128  # 16 k-chunks per batch
NQT = S // 512  # 4 q-tiles per batch

_BF16 = ml_dtypes.bfloat16

_cached_nc = None


# ---------------------------------------------------------------------------
# toolchain patches (walrus in this image accepts one sync-wait per inst)
# ---------------------------------------------------------------------------


def _apply_tile_patches():
    import re

    import bass_rust
    import concourse.mybir as mybir
    import concourse.tile as tile
    from concourse.vector_clock import ScopedClock

    if getattr(tile.TileContext, "_wait_patch_applied", False):
        return

    def _drain_and_barrier(self, tick_clock, wait_clock):
        ticks = list(map(int, re.findall(r"\d+", repr(tick_clock.global_clock))))
        nonzero = [(i, t) for i, t in enumerate(ticks) if t > 0]
        for i, t in nonzero:
            vc = bass_rust.VectorClock()
            vc.require_at_least(i, t)
            nop = self.nc.sync.nop(nofuse=True, hint=f"tail_wait_{i}")
            wait_clock.add_sem_waits(nop.ins, ScopedClock({None: vc}))
        self.nc.sync.drain()
        self.nc.all_engine_barrier()
        assert self.sems is not None
        popped = self.nc._tile_sem_poison_stack.pop()
        assert popped is self._sem_poison
        self.nc.clear_and_free_semaphores(list(self.sems.allocated().values()))
        self.nc.all_engine_barrier()

    tile.TileContext._drain_and_barrier = _drain_and_barrier
    tile.TileContext._wait_patch_applied = True


def _split_multi_waits(nc):
    """Hoist extra semaphore waits onto same-engine NoOps inserted before the
    instruction (engine streams execute in order, so this is equivalent)."""
    import bass_rust
    import concourse.mybir as mybir

    n_split = 0
    for f in nc.m.functions:
        for blk in f.blocks:
            insts = blk.instructions
            out = []
            changed = False
            for inst in insts:
                si = inst.sync_info
                if si is not None and len(si.on_wait) > 1:
                    keep, hoist = [], []
                    for w in si.on_wait:
                        (hoist if w.sync_type == "semaphore" else keep).append(w)
                    if hoist:
                        keep = keep + [hoist.pop()]
                    for w in hoist:
                        n_split += 1
                        nop = mybir.InstNoOp(
                            name=f"I-waitsplit-{n_split}", ins=[], outs=[]
                        )
                        nop.engine = inst.engine
                        nop.sync_info = bass_rust.SyncInfo(on_wait=[w], on_update=[])
                        out.append(nop)
                        changed = True
                    inst.sync_info = bass_rust.SyncInfo(
                        on_wait=keep, on_update=list(si.on_update)
                    )
                out.append(inst)
            if changed:
                blk.instructions = out
    return n_split


# ---------------------------------------------------------------------------
# device kernel
# ---------------------------------------------------------------------------


def _build_nc():
    import concourse.bass as bass
    import concourse.mybir as mybir
    import concourse.tile as tile

    _apply_tile_patches()

    BF = mybir.dt.bfloat16
    F32 = mybir.dt.float32
    ds = bass.ds

    nc = bass.Bass(num_devices=NCORES)

    xt = nc.declare_dram_parameter("xt", [D, T], BF, isOutput=False)
    wqkvt = nc.declare_dram_parameter("wqkvt", [D, 384], BF, isOutput=False)
    wot = nc.declare_dram_parameter("wot", [D, D], BF, isOutput=False)
    cosp = nc.declare_dram_parameter("cosp", [128, T], BF, isOutput=False)
    sinp = nc.declare_dram_parameter("sinp", [128, T], BF, isOutput=False)
    masks = nc.declare_dram_parameter("masks", [128, 4, 512], BF, isOutput=False)
    ident = nc.declare_dram_parameter("ident", [128, 64], BF, isOutput=False)
    ones64 = nc.declare_dram_parameter("ones64", [1, 64], BF, isOutput=False)
    yt = nc.declare_dram_parameter("yt", [D, TSH], F32, isOutput=True)

    Exp = mybir.ActivationFunctionType.Exp

    with tile.TileContext(nc) as tc:
        with (
            tc.tile_pool(name="persist", bufs=1) as pp,
            tc.tile_pool(name="dram", bufs=1, space="DRAM") as dram,
        ):
            # ---- persistent SBUF state ----
            wqkv_sb = pp.tile([128, 16, 384], BF, name="wqkv_sb")
            nc.sync.dma_start(
                wqkv_sb[:], wqkvt.rearrange("(c p) o -> p c o", p=128)
            )
            mask_sb = pp.tile([128, 4, 512], BF, name="mask_sb")
            nc.sync.dma_start(mask_sb[:], masks[:])
            id_sb = pp.tile([128, 64], BF, name="id_sb")
            nc.sync.dma_start(id_sb[:], ident[:])
            on_sb = pp.tile([1, 64], BF, name="on_sb")
            nc.sync.dma_start(on_sb[:], ones64[:])

            # q tiles (2 x 2 heads; projected then rotated in place) +
            # duplicated rotated k
            q_rot = [pp.tile([128, T], BF, name=f"q_rot{p}") for p in range(2)]
            k_rot = pp.tile([128, T], BF, name="k_rot")
            # v in natural layout [kt, 65] per batch (col 64 = ones)
            vnat = [pp.tile([128, NKT, 65], BF, name=f"vnat{b}") for b in range(B)]
            # unnormalized PV outputs + denominators
            pvu = pp.tile([64, 32, 512], BF, name="pvu")
            den = pp.tile([32, 512], F32, name="den")

            # input: 8 chunks (one per dest rank) x 256 local head-dims x 512 t
            a2a_in = dram.tile([NCORES, 4 * HD, TSH], BF, name="a2a_in")
            # output [2048, 512]: rows = 8 source ranks x 256 head-dims
            a2a_out = dram.tile([D, TSH], BF, name="a2a_out")

            # ---- phase 1: fused QKV projection (outputs transposed) ----
            with (
                tc.tile_pool(name="p12", bufs=1) as p12,
                tc.tile_pool(name="p1x", bufs=2) as p1x,
                tc.tile_pool(name="p1ps", bufs=4, space="PSUM") as p1ps,
            ):
                cos_sb = p12.tile([128, T], BF, name="cos_sb")
                sin_sb = p12.tile([128, T], BF, name="sin_sb")
                nc.sync.dma_start(cos_sb[:], cosp[:])
                nc.sync.dma_start(sin_sb[:], sinp[:])
                kv_raw = p12.tile([128, T], BF, name="kv_raw")
                dest = [q_rot[0], q_rot[1], kv_raw]
                for tt in range(T // 512):
                    xt_t = p1x.tile([128, 16, 512], BF, tag="xt_t", name="xt_t")
                    nc.sync.dma_start(
                        xt_t[:],
                        xt[:, ds(tt * 512, 512)].rearrange(
                            "(c p) t -> p c t", p=128
                        ),
                    )
                    for oc in range(3):
                        ps = p1ps.tile([128, 512], F32, tag="proj", name="proj_ps")
                        for kc in range(16):
                            nc.tensor.matmul(
                                ps[:],
                                lhsT=wqkv_sb[:, kc, ds(oc * 128, 128)],
                                rhs=xt_t[:, kc, :],
                                start=(kc == 0),
                                stop=(kc == 15),
                            )
                        nc.scalar.copy(
                            out=dest[oc][:, ds(tt * 512, 512)], in_=ps[:]
                        )

                # ---- phase 2a: rope q in place, rope k then duplicate ----
                for src in (q_rot[0], q_rot[1]):
                    sw = p12.tile([128, T], BF, tag="swap", name="swap", bufs=1)
                    nc.sync.dma_start(sw[0:32, :], src[32:64, :])
                    nc.sync.dma_start(sw[32:64, :], src[0:32, :])
                    nc.sync.dma_start(sw[64:96, :], src[96:128, :])
                    nc.sync.dma_start(sw[96:128, :], src[64:96, :])
                    t1 = p12.tile([128, T], BF, tag="ropetmp", name="ropetmp", bufs=1)
                    nc.vector.tensor_mul(t1[:], src[:], cos_sb[:])
                    nc.vector.tensor_mul(sw[:], sw[:], sin_sb[:])
                    nc.vector.tensor_add(src[:], t1[:], sw[:])
                # k: rope the 64 raw rows into k_rot[0:64], then duplicate
                ksw = p12.tile([128, T], BF, tag="swap", name="ksw", bufs=1)
                nc.sync.dma_start(ksw[0:32, :], kv_raw[32:64, :])
                nc.sync.dma_start(ksw[32:64, :], kv_raw[0:32, :])
                kt1 = p12.tile([128, T], BF, tag="ropetmp", name="kt1", bufs=1)
                nc.vector.tensor_mul(kt1[0:64, :], kv_raw[0:64, :], cos_sb[0:64, :])
                nc.vector.tensor_mul(ksw[0:64, :], ksw[0:64, :], sin_sb[0:64, :])
                nc.vector.tensor_add(k_rot[0:64, :], kt1[0:64, :], ksw[0:64, :])
                nc.sync.dma_start(k_rot[64:128, :], k_rot[0:64, :])

                # ---- phase 2b: v to natural layout with ones column ----
                with tc.tile_pool(name="p2ps", bufs=4, space="PSUM") as p2ps:
                    for b in range(B):
                        nc.gpsimd.memset(vnat[b][:], 1.0)
                        for c in range(NKT):
                            pt = p2ps.tile([128, 64], BF, tag="vt", name="vt_ps")
                            nc.tensor.transpose(
                                pt[:],
                                kv_raw[64:128, ds(b * S + c * 128, 128)],
                                id_sb[64:128, :],
                            )
                            nc.vector.tensor_copy(
                                out=vnat[b][:, c, 0:64], in_=pt[:]
                            )

            # ---- phase 3: attention (scores transposed, paired heads) ----
            # prefetch the first half of wo (n cols 0:1024) while attention runs
            wot1 = pp.tile([128, 16, D // 2], BF, name="wot1")
            nc.sync.dma_start(
                wot1[:],
                wot[:, 0 : D // 2].rearrange("(c p) n -> p c n", p=128),
            )

            with (
                tc.tile_pool(name="p3", bufs=8) as p3,
                tc.tile_pool(name="p3st", bufs=4) as p3st,
                tc.tile_pool(name="qkps", bufs=3, space="PSUM") as qkps,
                tc.tile_pool(name="pvps", bufs=4, space="PSUM") as pvps,
                tc.tile_pool(name="bcps", bufs=1, space="PSUM") as bcps,
            ):
                ridx = 0
                rmap = {}
                for p in range(2):
                    for b in range(B):
                        for qi in range(NQT):
                            nkt = 4 * qi + 4
                            qsl = ds(b * S + qi * 512, 512)
                            pv0 = pvps.tile([128, 512], F32, tag="pv", name="pv0")
                            pv1 = pvps.tile([128, 512], F32, tag="pv", name="pv1")
                            for kt in range(nkt):
                                ksl = ds(b * S + kt * 128, 128)
                                qkA = qkps.tile([128, 512], F32, tag="qk", name="qkA")
                                qkB = qkps.tile([128, 512], F32, tag="qk", name="qkB")
                                nc.tensor.matmul(
                                    qkA[:],
                                    lhsT=k_rot[0:64, ksl],
                                    rhs=q_rot[p][0:64, qsl],
                                    start=True,
                                    stop=True,
                                    tile_position=(0, 0),
                                )
                                nc.tensor.matmul(
                                    qkB[:],
                                    lhsT=k_rot[64:128, ksl],
                                    rhs=q_rot[p][64:128, qsl],
                                    start=True,
                                    stop=True,
                                    tile_position=(64, 0),
                                )
                                pA = p3.tile([128, 512], BF, tag="prob", name="pA")
                                pB = p3.tile([128, 512], BF, tag="prob", name="pB")
                                nc.scalar.activation(pA[:], qkA[:], Exp, scale=0.125)
                                nc.scalar.activation(pB[:], qkB[:], Exp, scale=0.125)
                                if kt >= 4 * qi:
                                    di = kt - 4 * qi
                                    nc.vector.tensor_mul(
                                        pA[:], pA[:], mask_sb[:, di, :]
                                    )
                                    nc.vector.tensor_mul(
                                        pB[:], pB[:], mask_sb[:, di, :]
                                    )
                                nc.tensor.matmul(
                                    pv0[0:65, :],
                                    lhsT=vnat[b][:, kt, :],
                                    rhs=pA[:],
                                    start=(kt == 0),
                                    stop=(kt == nkt - 1),
                                )
                                nc.tensor.matmul(
                                    pv1[0:65, :],
                                    lhsT=vnat[b][:, kt, :],
                                    rhs=pB[:],
                                    start=(kt == 0),
                                    stop=(kt == nkt - 1),
                                )
                            for hh, pv in ((0, pv0), (1, pv1)):
                                r = ridx
                                ridx += 1
                                rmap[r] = (b, p, qi, hh)
                                st = p3st.tile(
                                    [65, 512], F32, tag="dstage", name="dstage"
                                )
                                nc.scalar.copy(
                                    out=st[64:65, :], in_=pv[64:65, :]
                                )
                                nc.sync.dma_start(
                                    den[r : r + 1, :], st[64:65, :]
                                )
                                nc.vector.tensor_copy(
                                    out=pvu[:, r, :], in_=pv[0:64, :]
                                )

                # ---- phase 3.5: normalize + scatter into a2a_in ----
                rec = p3st.tile([32, 512], F32, name="rec", bufs=1)
                nc.vector.reciprocal(rec[:], den[:])
                recb = p3st.tile([32, 512], BF, name="recb", bufs=1)
                nc.vector.tensor_copy(recb[:], rec[:])
                for r in range(32):
                    b, p, qi, hh = rmap[r]
                    j = 4 * b + qi
                    r1 = p3st.tile([1, 512], BF, tag="r1", name="r1")
                    nc.sync.dma_start(r1[:], recb[r : r + 1, :])
                    bc_ps = bcps.tile([64, 512], F32, tag="bc", name="bc_ps")
                    nc.tensor.matmul(
                        bc_ps[:], lhsT=on_sb[:], rhs=r1[:], start=True, stop=True
                    )
                    ob = p3st.tile([64, 512], BF, tag="ob", name="ob")
                    nc.vector.tensor_mul(ob[:], pvu[:, r, :], bc_ps[:])
                    nc.sync.dma_start(
                        a2a_in[j, ds((2 * p + hh) * 64, 64), :], ob[:]
                    )

            # ---- phase 4: all-to-all + output projection ----
            nc.gpsimd.collective_compute(
                "AllToAll",
                mybir.AluOpType.bypass,
                replica_groups=[list(range(NCORES))],
                ins=[a2a_in[:].opt()],
                outs=[a2a_out[:].opt()],
            )

            with (
                tc.tile_pool(name="p4", bufs=4) as p4,
                tc.tile_pool(name="p4ps", bufs=4, space="PSUM") as p4ps,
            ):
                wot2 = p4.tile([128, 16, D // 2], BF, name="wot2", bufs=1)
                nc.sync.dma_start(
                    wot2[:],
                    wot[:, D // 2 : D].rearrange("(c p) n -> p c n", p=128),
                )
                of_sb = p4.tile([128, 16, TSH], BF, name="of_sb", bufs=1)
                nc.sync.dma_start(
                    of_sb[:], a2a_out.rearrange("(c p) t -> p c t", p=128)
                )
                for ncol in range(16):
                    wsrc = wot1 if ncol < 8 else wot2
                    coff = ncol * 128 if ncol < 8 else (ncol - 8) * 128
                    ps = p4ps.tile([128, 512], F32, tag="y", name="y_ps")
                    for mc in range(16):
                        nc.tensor.matmul(
                            ps[:],
                            lhsT=wsrc[:, mc, ds(coff, 128)],
                            rhs=of_sb[:, mc, :],
                            start=(mc == 0),
                            stop=(mc == 15),
                        )
                    ys = p4.tile([128, 512], F32, tag="yout", name="ys")
                    nc.vector.tensor_copy(out=ys[:], in_=ps[:])
                    nc.sync.dma_start(yt[ds(ncol * 128, 128), :], ys[:])

    _split_multi_waits(nc)
    return nc


def _get_nc():
    global _cached_nc
    if _cached_nc is None:
        _cached_nc = _build_nc()
    return _cached_nc


# ---------------------------------------------------------------------------
# host side
# ---------------------------------------------------------------------------


def _prep_inputs(x, freqs_cos, freqs_sin, wq, wk, wv, wo):
    """Build the 8 per-core input maps (bf16, pre-transposed/permuted)."""
    perm = np.concatenate([np.arange(0, HD, 2), np.arange(1, HD, 2)])

    xt = np.ascontiguousarray(
        x.reshape(T, D).T.astype(_BF16)
    )  # [D, T]
    wot = np.ascontiguousarray(wo.T.astype(_BF16))  # [D, D]

    cosT = freqs_cos.T.astype(np.float32)  # [32, S]
    sinT = freqs_sin.T.astype(np.float32)
    cos2 = np.concatenate([cosT, cosT], axis=1)  # [32, T]
    sin2 = np.concatenate([sinT, sinT], axis=1)
    cosp = np.tile(cos2, (4, 1)).astype(_BF16)  # [128, T]
    sinp = np.concatenate([-sin2, sin2, -sin2, sin2], axis=0).astype(_BF16)

    # masks[p, d, j] = 1 if (p + 128 d) <= j  (valid kt <= qt)
    i = np.arange(128)[:, None, None]
    d = np.arange(4)[None, :, None]
    j = np.arange(512)[None, None, :]
    masks = ((i + 128 * d) <= j).astype(_BF16)

    ident = np.concatenate([np.eye(64), np.eye(64)], axis=0).astype(_BF16)
    ones64 = np.ones((1, HD), dtype=_BF16)

    shared = {
        "xt": xt,
        "wot": wot,
        "cosp": cosp,
        "sinp": sinp,
        "masks": masks,
        "ident": ident,
        "ones64": ones64,
    }

    in_maps = []
    for c in range(NCORES):
        rows = []
        for hl in range(4):  # local q heads
            base = (4 * c + hl) * HD
            rows.append(wq[base + perm, :])
        kbase = c * HD
        rows.append(wk[kbase + perm, :])
        rows.append(wv[kbase : kbase + HD, :])
        wqkv = np.concatenate(rows, axis=0)  # [384, D]
        wqkvt = np.ascontiguousarray(wqkv.T.astype(_BF16))  # [D, 384]
        in_maps.append({**shared, "wqkvt": wqkvt})
    return in_maps


def _run(in_maps, trace=False):
    from concourse.bass_utils import run_bass_kernel_spmd

    nc = _get_nc()
    return run_bass_kernel_spmd(
        nc, in_maps, core_ids=list(range(NCORES)), trace=trace
    )


def kernel(x, freqs_cos, freqs_sin, wq, wk, wv, wo, _trace=False):
    x = np.asarray(x, dtype=np.float32)
    in_maps = _prep_inputs(
        np.asarray(x, np.float32),
        np.asarray(freqs_cos, np.float32),
        np.asarray(freqs_sin, np.float32),
        np.asarray(wq, np.float32),
        np.asarray(wk, np.float32),
        np.asarray(wv, np.float32),
        np.asarray(wo, np.float32),
    )
    res = _run(in_maps, trace=_trace)
    y = np.empty((T, D), dtype=np.float32)
    for c in range(NCORES):
        y[c * TSH : (c + 1) * TSH, :] = res.results[c]["yt"].T
    out = y.reshape(B, S, D)
    if _trace:
        kernel.last_exec_time_ns = res.exec_time_ns
        kernel.last_results = res
    return out


# revision 13
# speedup vs baseline: 1.0681x; 1.0681x over previous
"""Distributed Trainium2 kernel for causal GQA attention (llama-style).

Reference computation (B=2, S=2048, D=2048, H=32 q-heads, KV=8 kv-heads,
HD=64, f32):
    q = rope(x @ wq.T), k = rope(x @ wk.T), v = x @ wv.T
    out = causal_softmax(q k^T / 8) v        (GQA: 4 q-heads per kv head)
    y = out @ wo.T

Sharding: tensor-parallel over heads — core c owns q-heads [4c, 4c+4) and
kv-head c, so each KV group stays local. The attention output is exchanged
with AllToAll (scatter tokens / gather heads), after which each core runs
the full wo projection on its 512-token shard. The host concatenates the
8 token shards — no all-reduce anywhere.

Device-side structure (per core, all phases software-pipelined per batch):
  proj(b0) -> rope(b0) -> [proj(b1) || attn(b0)] -> rope(b1)
  -> [A2A(b0) || attn(b1)] -> A2A(b1) -> wo projection
The two A2As are zero-padded per-batch halves written into separate
output buffers that are summed before the wo matmul, keeping the SPMD
program rank-independent.

Tricks:
  - All inputs pre-cast to bf16 and pre-arranged on the host so every big
    DMA is contiguous per partition; x is passed as x^T so the
    contraction dim lands on SBUF partitions.
  - RoPE: wq/wk rows are permuted per-head on the host (even indices
    first), turning interleaved rope into rotate-half form = two
    tensor-tensor multiplies + add with precomputed [128, T] cos/sin
    tables (the 32-row block swap is done with SBUF-to-SBUF DMAs).
  - Scores are computed transposed (S^T[kt, qt]) so the PV matmul needs
    no transposes; the two heads of a 128-row q tile are computed
    concurrently with row-tiled matmuls (tile_position (0,0)/(64,0))
    against a duplicated [k; k] tile.
  - Softmax runs without max-subtraction (scores are O(5) by
    construction); causality is structural (upper tiles skipped) plus a
    binary mask multiply on diagonal tiles after exp.
  - The softmax denominator falls out of the PV matmul via a ones column
    appended to v (M=65); normalization is a batched reciprocal plus
    rank-1 broadcast matmuls.
"""

import sys

if "/opt/trn_rl_repo" not in sys.path:
    sys.path.insert(0, "/opt/trn_rl_repo")

import numpy as np
import ml_dtypes

B, S, D = 2, 2048, 2048
H, KV, HD = 32, 8, 64
NCORES = 8
T = B * S  # 4096 flattened tokens
TSH = T // NCORES  # 512-token output shard per core
NKT = S // 128  # 16 k-chunks per batch
NQT = S // 512  # 4 q-tiles per batch

_BF16 = ml_dtypes.bfloat16

_cached_nc = None


# ---------------------------------------------------------------------------
# toolchain patches (walrus in this image accepts one sync-wait per inst)
# ---------------------------------------------------------------------------


def _apply_tile_patches():
    import re

    import bass_rust
    import concourse.tile as tile
    from concourse.vector_clock import ScopedClock

    if getattr(tile.TileContext, "_wait_patch_applied", False):
        return

    def _drain_and_barrier(self, tick_clock, wait_clock):
        ticks = list(map(int, re.findall(r"\d+", repr(tick_clock.global_clock))))
        for i, t in [(i, t) for i, t in enumerate(ticks) if t > 0]:
            vc = bass_rust.VectorClock()
            vc.require_at_least(i, t)
            nop = self.nc.sync.nop(nofuse=True, hint=f"tail_wait_{i}")
            wait_clock.add_sem_waits(nop.ins, ScopedClock({None: vc}))
        self.nc.sync.drain()
        self.nc.all_engine_barrier()
        assert self.sems is not None
        popped = self.nc._tile_sem_poison_stack.pop()
        assert popped is self._sem_poison
        self.nc.clear_and_free_semaphores(list(self.sems.allocated().values()))
        self.nc.all_engine_barrier()

    tile.TileContext._drain_and_barrier = _drain_and_barrier
    tile.TileContext._wait_patch_applied = True


def _split_multi_waits(nc):
    """Hoist extra semaphore waits onto same-engine NoOps inserted before the
    instruction (engine streams execute in order, so this is equivalent)."""
    import bass_rust
    import concourse.mybir as mybir

    n_split = 0
    for f in nc.m.functions:
        for blk in f.blocks:
            insts = blk.instructions
            out = []
            changed = False
            for inst in insts:
                si = inst.sync_info
                if si is not None and len(si.on_wait) > 1:
                    keep, hoist = [], []
                    for w in si.on_wait:
                        (hoist if w.sync_type == "semaphore" else keep).append(w)
                    if hoist:
                        keep = keep + [hoist.pop()]
                    for w in hoist:
                        n_split += 1
                        nop = mybir.InstNoOp(
                            name=f"I-waitsplit-{n_split}", ins=[], outs=[]
                        )
                        nop.engine = inst.engine
                        nop.sync_info = bass_rust.SyncInfo(on_wait=[w], on_update=[])
                        out.append(nop)
                        changed = True
                    inst.sync_info = bass_rust.SyncInfo(
                        on_wait=keep, on_update=list(si.on_update)
                    )
                out.append(inst)
            if changed:
                blk.instructions = out
    return n_split


# ---------------------------------------------------------------------------
# device kernel
# ---------------------------------------------------------------------------


def _build_nc():
    import concourse.bass as bass
    import concourse.mybir as mybir
    import concourse.tile as tile

    _apply_tile_patches()

    BF = mybir.dt.bfloat16
    F32 = mybir.dt.float32
    ds = bass.ds
    Exp = mybir.ActivationFunctionType.Exp

    nc = bass.Bass(num_devices=NCORES)

    # host-prearranged [partition, chunk, free] layouts -> contiguous DMAs
    xt = nc.declare_dram_parameter("xt", [128, 8, 16, 512], BF, isOutput=False)
    wqkvt = nc.declare_dram_parameter("wqkvt", [128, 16, 384], BF, isOutput=False)
    wotq = [
        nc.declare_dram_parameter(f"wotq{q}", [128, 16, 512], BF, isOutput=False)
        for q in range(4)
    ]
    cosp = nc.declare_dram_parameter("cosp", [128, T], BF, isOutput=False)
    sinp = nc.declare_dram_parameter("sinp", [128, T], BF, isOutput=False)
    masks = nc.declare_dram_parameter("masks", [128, 4, 512], BF, isOutput=False)
    ident = nc.declare_dram_parameter("ident", [128, 64], BF, isOutput=False)
    ones64 = nc.declare_dram_parameter("ones64", [1, 64], BF, isOutput=False)
    yt = nc.declare_dram_parameter("yt", [D, TSH], F32, isOutput=True)

    with tile.TileContext(nc) as tc:
        with (
            tc.tile_pool(name="persist", bufs=1) as pp,
            tc.tile_pool(name="ps", bufs=1, space="PSUM") as psp,
            tc.tile_pool(name="dram", bufs=1, space="DRAM") as dram,
        ):
            wkB_cm = tc.tile_pool(name="wkB", bufs=1)
            wkB = wkB_cm.__enter__()
            wkA_cm = tc.tile_pool(name="wkA", bufs=1)
            wkA = wkA_cm.__enter__()
            # ---- persistent SBUF state ----
            wqkv_sb = pp.tile([128, 16, 384], BF, name="wqkv_sb")
            nc.sync.dma_start(wqkv_sb[:], wqkvt[:])
            mask_sb = pp.tile([128, 4, 512], BF, name="mask_sb")
            nc.sync.dma_start(mask_sb[:], masks[:])
            id_sb = pp.tile([128, 64], BF, name="id_sb")
            nc.sync.dma_start(id_sb[:], ident[:])
            on_sb = pp.tile([1, 64], BF, name="on_sb")
            nc.sync.dma_start(on_sb[:], ones64[:])
            q_rot = [pp.tile([128, T], BF, name=f"q_rot{p}") for p in range(2)]
            k_rot = pp.tile([128, T], BF, name="k_rot")
            vnat = [pp.tile([128, NKT, 65], BF, name=f"vnat{b}") for b in range(B)]
            pvu = pp.tile([64, 32, 512], BF, name="pvu")
            den = [pp.tile([16, 512], F32, name=f"den{b}") for b in range(B)]

            cos_sb = wkA.tile([128, T], BF, name="cos_sb")
            sin_sb = wkA.tile([128, T], BF, name="sin_sb")
            nc.sync.dma_start(cos_sb[:], cosp[:])
            nc.sync.dma_start(sin_sb[:], sinp[:])
            kv_raw = wkA.tile([128, T], BF, name="kv_raw")

            # a2a buffers: one zero-padded pair per batch
            a2a_in = [
                dram.tile([NCORES, 4 * HD, TSH], BF, name=f"a2a_in{b}")
                for b in range(B)
            ]
            a2a_out = [
                dram.tile([D, TSH], BF, name=f"a2a_out{b}") for b in range(B)
            ]
            # zero the four invalid chunks of each a2a input half
            zs = wkA.tile([128, 2048], BF, name="zs")
            nc.gpsimd.memset(zs[:], 0.0)
            zsv = zs.rearrange("p (jc t) -> p jc t", t=512)
            for lo, hi, buf in ((4, 6, 0), (6, 8, 0), (0, 2, 1), (2, 4, 1)):
                nc.sync.dma_start(
                    a2a_in[buf][lo:hi].rearrange("j (c p) t -> p (j c) t", p=128),
                    zsv,
                )

            def proj_batch(b):
                dest = [q_rot[0], q_rot[1], kv_raw]
                for tt in range(4 * b, 4 * b + 4):
                    xt_t = wkA.tile([128, 16, 512], BF, tag="xt_t", name="xt_t", bufs=2)
                    nc.sync.dma_start(xt_t[:], xt[:, tt])
                    for oc in range(3):
                        ps = psp.tile([128, 512], F32, tag="proj", name="proj_ps", bufs=2)
                        for kc in range(16):
                            nc.tensor.matmul(
                                ps[:],
                                lhsT=wqkv_sb[:, kc, ds(oc * 128, 128)],
                                rhs=xt_t[:, kc, :],
                                start=(kc == 0),
                                stop=(kc == 15),
                            )
                        nc.scalar.copy(
                            out=dest[oc][:, ds(tt * 512, 512)], in_=ps[:]
                        )

            def rope_batch(b):
                sl = ds(b * S, S)
                for src in (q_rot[0], q_rot[1]):
                    sw = wkA.tile([128, S], BF, tag="swap", name="swap", bufs=1)
                    nc.sync.dma_start(sw[0:32, :], src[32:64, sl])
                    nc.sync.dma_start(sw[32:64, :], src[0:32, sl])
                    nc.sync.dma_start(sw[64:96, :], src[96:128, sl])
                    nc.sync.dma_start(sw[96:128, :], src[64:96, sl])
                    t1 = wkA.tile([128, S], BF, tag="ropetmp", name="ropetmp", bufs=1)
                    nc.vector.tensor_mul(t1[:], src[:, sl], cos_sb[:, sl])
                    nc.vector.tensor_mul(sw[:], sw[:], sin_sb[:, sl])
                    nc.vector.tensor_add(src[:, sl], t1[:], sw[:])
                ksw = wkA.tile([128, S], BF, tag="swap", name="ksw", bufs=1)
                nc.sync.dma_start(ksw[0:32, :], kv_raw[32:64, sl])
                nc.sync.dma_start(ksw[32:64, :], kv_raw[0:32, sl])
                kt1 = wkA.tile([128, S], BF, tag="ropetmp", name="kt1", bufs=1)
                nc.vector.tensor_mul(kt1[0:64, :], kv_raw[0:64, sl], cos_sb[0:64, sl])
                nc.vector.tensor_mul(ksw[0:64, :], ksw[0:64, :], sin_sb[0:64, sl])
                nc.vector.tensor_add(k_rot[0:64, sl], kt1[0:64, :], ksw[0:64, :])
                nc.sync.dma_start(k_rot[64:128, sl], k_rot[0:64, sl])
                # v -> natural layout with trailing ones column
                nc.gpsimd.memset(vnat[b][:], 1.0)
                for c in range(NKT):
                    pt = psp.tile([128, 64], BF, tag="bcvt", name="vt_ps", bufs=1)
                    nc.tensor.transpose(
                        pt[:],
                        kv_raw[64:128, ds(b * S + c * 128, 128)],
                        id_sb[64:128, :],
                    )
                    nc.vector.tensor_copy(out=vnat[b][:, c, 0:64], in_=pt[:])

            def attn_batch(b):
                for p in range(2):
                    for qi in range(NQT):
                        nkt = 4 * qi + 4
                        qsl = ds(b * S + qi * 512, 512)
                        pv0 = psp.tile([128, 512], F32, tag="pv", name="pv0", bufs=2)
                        pv1 = psp.tile([128, 512], F32, tag="pv", name="pv1", bufs=2)
                        for kt in range(nkt):
                            ksl = ds(b * S + kt * 128, 128)
                            qkA = psp.tile([128, 512], F32, tag="qk", name="qkA", bufs=3)
                            qkB = psp.tile([128, 512], F32, tag="qk", name="qkB", bufs=3)
                            nc.tensor.matmul(
                                qkA[:],
                                lhsT=k_rot[0:64, ksl],
                                rhs=q_rot[p][0:64, qsl],
                                start=True,
                                stop=True,
                                tile_position=(0, 0),
                            )
                            nc.tensor.matmul(
                                qkB[:],
                                lhsT=k_rot[64:128, ksl],
                                rhs=q_rot[p][64:128, qsl],
                                start=True,
                                stop=True,
                                tile_position=(64, 0),
                            )
                            pA = wkB.tile([128, 512], BF, tag="prob", name="pA", bufs=8)
                            pB = wkB.tile([128, 512], BF, tag="prob", name="pB", bufs=8)
                            nc.scalar.activation(pA[:], qkA[:], Exp, scale=0.125)
                            nc.scalar.activation(pB[:], qkB[:], Exp, scale=0.125)
                            if kt >= 4 * qi:
                                di = kt - 4 * qi
                                nc.vector.tensor_mul(pA[:], pA[:], mask_sb[:, di, :])
                                nc.vector.tensor_mul(pB[:], pB[:], mask_sb[:, di, :])
                            nc.tensor.matmul(
                                pv0[0:65, :],
                                lhsT=vnat[b][:, kt, :],
                                rhs=pA[:],
                                start=(kt == 0),
                                stop=(kt == nkt - 1),
                            )
                            nc.tensor.matmul(
                                pv1[0:65, :],
                                lhsT=vnat[b][:, kt, :],
                                rhs=pB[:],
                                start=(kt == 0),
                                stop=(kt == nkt - 1),
                            )
                        for hh, pv in ((0, pv0), (1, pv1)):
                            r = b * 16 + (2 * p + hh) * 4 + qi
                            st = wkB.tile([65, 512], F32, tag="dstage", name="dstage", bufs=4)
                            nc.scalar.copy(out=st[64:65, :], in_=pv[64:65, :])
                            rl = (2 * p + hh) * 4 + qi
                            nc.sync.dma_start(den[b][rl : rl + 1, :], st[64:65, :])
                            nc.vector.tensor_copy(out=pvu[:, r, :], in_=pv[0:64, :])

            def norm_a2a_batch(b):
                rec = wkB.tile([16, 512], F32, tag="rec", name="rec", bufs=2)
                nc.vector.reciprocal(rec[:], den[b][:])
                recb = wkB.tile([16, 512], BF, tag="recb", name="recb", bufs=2)
                nc.vector.tensor_copy(recb[:], rec[:])
                for p in range(2):
                    for hh in range(2):
                        for qi in range(NQT):
                            r = b * 16 + (2 * p + hh) * 4 + qi
                            rl = (2 * p + hh) * 4 + qi
                            r1 = wkB.tile([1, 512], BF, tag="r1", name="r1", bufs=4)
                            nc.sync.dma_start(r1[:], recb[rl : rl + 1, :])
                            bc_ps = psp.tile([64, 512], F32, tag="bcvt", name="bc_ps", bufs=1)
                            nc.tensor.matmul(
                                bc_ps[:], lhsT=on_sb[:], rhs=r1[:],
                                start=True, stop=True,
                            )
                            ob = wkB.tile([64, 512], BF, tag="ob", name="ob", bufs=4)
                            nc.vector.tensor_mul(ob[:], pvu[:, r, :], bc_ps[:])
                            nc.sync.dma_start(
                                a2a_in[b][4 * b + qi, ds((2 * p + hh) * 64, 64), :],
                                ob[:],
                            )
                nc.gpsimd.collective_compute(
                    "AllToAll",
                    mybir.AluOpType.bypass,
                    replica_groups=[list(range(NCORES))],
                    ins=[a2a_in[b][:].opt()],
                    outs=[a2a_out[b][:].opt()],
                )

            # ---- pipelined schedule ----
            proj_batch(0)
            rope_batch(0)
            proj_batch(1)  # PE work overlapping attn(0)'s ACT/DVE
            attn_batch(0)
            rope_batch(1)
            wkA_cm.__exit__(None, None, None)  # free proj/rope space
            wkC_cm = tc.tile_pool(name="wkC", bufs=1)
            wkC = wkC_cm.__enter__()
            norm_a2a_batch(0)  # A2A(0) hides under attn(1)
            attn_batch(1)
            norm_a2a_batch(1)

            # ---- output projection on this core's 512-token shard ----
            of0 = wkC.tile([128, 16, TSH], BF, name="of0", bufs=1)
            of1 = wkC.tile([128, 16, TSH], BF, name="of1", bufs=1)
            nc.sync.dma_start(of0[:], a2a_out[0].rearrange("(c p) t -> p c t", p=128))
            nc.sync.dma_start(of1[:], a2a_out[1].rearrange("(c p) t -> p c t", p=128))
            nc.vector.tensor_add(of0[:], of0[:], of1[:])
            for q in range(4):
                wq_sb = wkC.tile([128, 16, 512], BF, tag="wotq", name="wq_sb", bufs=2)
                nc.sync.dma_start(wq_sb[:], wotq[q][:])
                for nn in range(4):
                    ncol = q * 4 + nn
                    ps = psp.tile([128, 512], F32, tag="proj", name="y_ps", bufs=2)
                    for mc in range(16):
                        nc.tensor.matmul(
                            ps[:],
                            lhsT=wq_sb[:, mc, ds(nn * 128, 128)],
                            rhs=of0[:, mc, :],
                            start=(mc == 0),
                            stop=(mc == 15),
                        )
                    ys = wkC.tile([128, 512], F32, tag="yout", name="ys", bufs=4)
                    nc.vector.tensor_copy(out=ys[:], in_=ps[:])
                    nc.sync.dma_start(yt[ds(ncol * 128, 128), :], ys[:])
            wkC_cm.__exit__(None, None, None)
            wkB_cm.__exit__(None, None, None)

    _split_multi_waits(nc)
    return nc


def _get_nc():
    global _cached_nc
    if _cached_nc is None:
        _cached_nc = _build_nc()
    return _cached_nc


# ---------------------------------------------------------------------------
# host side
# ---------------------------------------------------------------------------


def _prep_inputs(x, freqs_cos, freqs_sin, wq, wk, wv, wo):
    """Build the 8 per-core input maps (bf16, pre-transposed/permuted)."""
    perm = np.concatenate([np.arange(0, HD, 2), np.arange(1, HD, 2)])

    # x^T [D, T] -> [128, 8, 16, 512]: (p, tt, kc, t) = xT[kc*128+p, tt*512+t]
    xT = x.reshape(T, D).T.astype(_BF16)
    xtarr = np.ascontiguousarray(
        xT.reshape(16, 128, 8, 512).transpose(1, 2, 0, 3)
    )

    woT = wo.T.astype(_BF16)  # [D(m), D(n)]
    wotq = [
        np.ascontiguousarray(
            woT[:, q * 512 : (q + 1) * 512]
            .reshape(16, 128, 512)
            .transpose(1, 0, 2)
        )
        for q in range(4)
    ]

    cosT = freqs_cos.T.astype(np.float32)  # [32, S]
    sinT = freqs_sin.T.astype(np.float32)
    cos2 = np.concatenate([cosT, cosT], axis=1)  # [32, T]
    sin2 = np.concatenate([sinT, sinT], axis=1)
    cosp = np.tile(cos2, (4, 1)).astype(_BF16)  # [128, T]
    sinp = np.concatenate([-sin2, sin2, -sin2, sin2], axis=0).astype(_BF16)

    # masks[p, d, j] = 1 if (p + 128 d) <= j   (valid kt <= qt)
    i = np.arange(128)[:, None, None]
    dd = np.arange(4)[None, :, None]
    j = np.arange(512)[None, None, :]
    masks = ((i + 128 * dd) <= j).astype(_BF16)

    ident = np.concatenate([np.eye(64), np.eye(64)], axis=0).astype(_BF16)
    ones64 = np.ones((1, HD), dtype=_BF16)

    shared = {
        "xt": xtarr,
        "wotq0": wotq[0],
        "wotq1": wotq[1],
        "wotq2": wotq[2],
        "wotq3": wotq[3],
        "cosp": cosp,
        "sinp": sinp,
        "masks": masks,
        "ident": ident,
        "ones64": ones64,
    }

    in_maps = []
    for c in range(NCORES):
        rows = []
        for hl in range(4):  # local q heads
            base = (4 * c + hl) * HD
            rows.append(wq[base + perm, :])
        kbase = c * HD
        rows.append(wk[kbase + perm, :])
        rows.append(wv[kbase : kbase + HD, :])
        wqkv = np.concatenate(rows, axis=0)  # [384, D]
        wqkvt = np.ascontiguousarray(
            wqkv.T.astype(_BF16).reshape(16, 128, 384).transpose(1, 0, 2)
        )
        in_maps.append({**shared, "wqkvt": wqkvt})
    return in_maps


def _run(in_maps, trace=False):
    from concourse.bass_utils import run_bass_kernel_spmd

    nc = _get_nc()
    return run_bass_kernel_spmd(
        nc, in_maps, core_ids=list(range(NCORES)), trace=trace
    )


def kernel(x, freqs_cos, freqs_sin, wq, wk, wv, wo, _trace=False):
    in_maps = _prep_inputs(
        np.asarray(x, np.float32),
        np.asarray(freqs_cos, np.float32),
        np.asarray(freqs_sin, np.float32),
        np.asarray(wq, np.float32),
        np.asarray(wk, np.float32),
        np.asarray(wv, np.float32),
        np.asarray(wo, np.float32),
    )
    res = _run(in_maps, trace=_trace)
    y = np.empty((T, D), dtype=np.float32)
    for c in range(NCORES):
        y[c * TSH : (c + 1) * TSH, :] = res.results[c]["yt"].T
    out = y.reshape(B, S, D)
    if _trace:
        kernel.last_exec_time_ns = res.exec_time_ns
        kernel.last_results = res
    return out
